# revision 12
# baseline (speedup 1.0000x reference)
"""Bass/Trainium2 kernel for nn_AttentionLayer (B=8, SQ=SV=2048, D=1024, fp32).

attention = softmax(Q @ V^T) @ V, per batch element.

Strategy (v3)
-------------
- Batch-parallel over 8 NeuronCores (1 batch element per core, no collectives).
- fp16 operands on TensorE (full rate), fp32 PSUM accumulation, softmax in
  fp32 (DVE row-max, ScalarE exp LUT with per-row bias + fused row-sum).
- Input staging minimizes prologue HBM traffic (the v2 bottleneck):
  * V: 8 half-wave SWDGE casts fp32->fp16 straight into SBUF (8MB HBM
    read, nothing else). V^T stripes are produced by PE-mode transposes
    (tensor.transpose via identity, ~75ns each) from SBUF while the PE
    is otherwise idle - V never round-trips through DRAM.
  * Q chunk 0: two SWDGE casts to SBUF + PE transposes (so the matmul
    stream can start at ~14us).
  * Q chunks 1-3: fp32 loads on the (idle) sync HWDGE ring, engine cast
    to fp16, then PE transposes emitted at the round boundary that
    consumes them - no DRAM image, no DMA transpose, no timing risk.
  * E (softmax numerator) transposed SBUF->SBUF per q-tile (no DRAM).
- stage1 emitted in (q-tile, s-chunk) units of 8 matmuls, round-of-4
  interleaved with the V^T transpose chunks so matmuls start as soon as
  the first chunk lands.
- All DMA transposes (eT only) ride the sync HWDGE ring exclusively.
- All matmuls converted to self-loading (explicit LDWEIGHTS stripped).
- Output scale+store split in half-tiles to shorten the drain tail.
"""

import sys

if "/opt/trn_rl_repo" not in sys.path:
    sys.path.insert(0, "/opt/trn_rl_repo")

import numpy as np

B, SQ, SV, D = 8, 2048, 2048, 1024
P = 128
N_CORES = 8


def _strip_all_ldweights(nc):
    """Remove every InstLdweights, migrating its semaphore waits/updates onto
    the next PE instruction (its paired InstMatmult)."""
    import concourse.mybir as mybir

    removed = 0
    for fn in nc.m.functions:
        for bb in fn.blocks:
            keep = []
            pending = []
            for inst in bb.instructions:
                if isinstance(inst, mybir.InstLdweights):
                    if inst.sync_info is not None:
                        pending.append(inst.sync_info)
                    removed += 1
                    continue
                if isinstance(inst, mybir.InstMatmult):
                    inst.ldweights = True
                    if pending:
                        waits, updates = [], []
                        for si in pending:
                            waits.extend(list(si.on_wait))
                            updates.extend(list(si.on_update))
                        mi = inst.sync_info
                        if mi is not None:
                            waits.extend(list(mi.on_wait))
                            updates.extend(list(mi.on_update))
                        inst.sync_info = mybir.SyncInfo(
                            on_wait=waits, on_update=updates
                        )
                        pending = []
                keep.append(inst)
            assert not pending, "dangling LDW sync with no following matmul"
            bb.instructions[:] = keep
    return removed


def build_attention_nc(sq=SQ, sv=SV, d=D, lag=5, rnd=4, compile=True):
    import concourse.bass as bass
    import concourse.mybir as mybir
    from concourse import bacc
    from concourse.tile import TileContext
    from concourse.masks import make_identity

    f32 = mybir.dt.float32
    f16 = mybir.dt.float16
    X = mybir.AxisListType.X
    Exp = mybir.ActivationFunctionType.Exp

    NQT, NST, NKT = sq // P, sv // P, d // P
    CH = 512                  # row chunk (cast/xpose granularity = psum chunk)
    NCH = sv // CH            # 4
    NJ = sv // CH
    DCH = 512
    NDCH = d // DCH
    TPC = CH // P             # tiles per chunk (4)

    nc = bacc.Bacc("TRN2", target_bir_lowering=False, debug=False)
    q = nc.dram_tensor("q", [sq, d], f32, kind="ExternalInput").ap()
    v = nc.dram_tensor("v", [sv, d], f32, kind="ExternalInput").ap()
    out = nc.dram_tensor("out", [sq, d], f32, kind="ExternalOutput").ap()

    with TileContext(nc) as tc:
        with (
            tc.tile_pool(name="dram", bufs=1, space="DRAM") as dram_pool,
            tc.tile_pool(name="resident", bufs=1) as res_pool,
            tc.tile_pool(name="qf32", bufs=1) as qf32_pool,
            tc.tile_pool(name="qf16", bufs=2) as qf16_pool,
            tc.tile_pool(name="ssb", bufs=rnd) as ssb_pool,
            tc.tile_pool(name="esb", bufs=2) as esb_pool,
            tc.tile_pool(name="etp", bufs=lag + 2) as et_pool,
            tc.tile_pool(name="osb", bufs=2) as osb_pool,
            tc.tile_pool(name="small", bufs=3 * (lag + 3)) as small_pool,
            tc.tile_pool(name="xpsum", bufs=2, space="PSUM") as xp_pool,
            tc.tile_pool(name="spsum", bufs=2, space="PSUM") as sp_pool,
            tc.tile_pool(name="opsum", bufs=2, space="PSUM") as opsum_pool,
        ):
            # Residents:
            vf_all = res_pool.tile([P, NST, d], f16, name="vf_all")
            vT_all = res_pool.tile([P, NCH, NKT, CH], f16, name="vT_all")
            qT_all = res_pool.tile([P, NCH, NKT, CH], f16, name="qT_all")
            ident = res_pool.tile([P, P], f16, name="ident")
            make_identity(nc, ident)

            # ---- SWDGE ring (gpsimd): interleaved Q-c0 + V half-wave casts
            def swdge_cast_rows(dst_sbuf, src_dram, r0, r1):
                src = src_dram[r0 * P : r1 * P, :].rearrange(
                    "(si p) d -> p si d", p=P
                )
                nc.gpsimd.dma_start(out=dst_sbuf, in_=src)

            qf16_c0 = qf16_pool.tile([P, TPC, d], f16, name="qf16")
            # ring order: qc0a, vh0, qc0b, vh1..vh7 - Q tile 0 and V chunk 0
            # land first so PE transposes + the first units start ASAP
            swdge_cast_rows(qf16_c0[:, 0:2, :], q, 0, 2)
            swdge_cast_rows(vf_all[:, 0:2, :], v, 0, 2)
            swdge_cast_rows(qf16_c0[:, 2:4, :], q, 2, 4)
            for h in range(1, 8):
                swdge_cast_rows(vf_all[:, 2 * h : 2 * h + 2, :], v, 2 * h, 2 * h + 2)

            # ---- PE transposes: src [128,128] f16 SBUF -> PSUM f16 --------
            # One PSUM tile gathers all 8 d-stripes of one 128-row tile;
            # a single strided DVE copy scatters them into the resident.
            def pe_xpose_tile(dst_all, c, r, src_tile_ap):
                ps = xp_pool.tile([P, NKT * P], f16, name="xpsum")
                for dk in range(NKT):
                    nc.tensor.transpose(
                        ps[:, dk * P : (dk + 1) * P],
                        src_tile_ap[:, dk * P : (dk + 1) * P],
                        ident,
                    )
                nc.vector.tensor_copy(
                    out=dst_all[:, c, :, r * P : (r + 1) * P],
                    in_=ps.rearrange("p (dk r) -> p dk r", dk=NKT),
                )

            # ---- sync ring: Q c1-3 fp32 loads + engine cast to fp16 -------
            # (PE transposes them at the round boundary that consumes them)
            qf16_chunks = {0: qf16_c0}
            for c in range(1, NCH):
                t32 = qf32_pool.tile([P, TPC, d], f32, name="qf32")
                src = q[c * CH : (c + 1) * CH, :].rearrange(
                    "(si p) d -> p si d", p=P
                )
                nc.sync.dma_start(out=t32, in_=src)
                t16 = qf16_pool.tile([P, TPC, d], f16, name="qf16")
                if c % 2 == 1:
                    nc.scalar.copy(t16, t32)
                else:
                    nc.vector.tensor_copy(out=t16, in_=t32)
                qf16_chunks[c] = t16

            # ---------------- main pipeline --------------------------------
            s_sb = {}
            state = {}
            done = []
            emitted2 = 0

            def unit(qi, j):
                sp = sp_pool.tile([P, CH], f32, name="spsum")
                cq, rq = qi // TPC, qi % TPC
                for dk in range(NKT):
                    nc.tensor.matmul(
                        sp,
                        lhsT=qT_all[:, cq, dk, rq * P : (rq + 1) * P],
                        rhs=vT_all[:, j, dk, :],
                        start=(dk == 0),
                        stop=(dk == NKT - 1),
                    )
                if qi not in s_sb:
                    s_sb[qi] = ssb_pool.tile([P, sv], f32, name="s_sb")
                dst = s_sb[qi][:, j * CH : (j + 1) * CH]
                if j % 2 == 0:
                    nc.scalar.copy(dst, sp)
                else:
                    nc.vector.tensor_copy(out=dst, in_=sp)

            def finish_stage1(qi):
                s = s_sb.pop(qi)
                negm = small_pool.tile([P, 1], f32, name="negm")
                nc.vector.reduce_max(negm, s, axis=X, negate=True)
                e_sb = esb_pool.tile([P, sv], f16, name="e_sb")
                lsum = small_pool.tile([P, 1], f32, name="lsum")
                nc.scalar.activation(
                    e_sb, s, Exp, bias=negm, scale=1.0, accum_out=lsum
                )
                r = small_pool.tile([P, 1], f32, name="r")
                nc.vector.reciprocal(r, lsum)
                eT = et_pool.tile([P, NST, P], f16, name="eT")
                nc.sync.dma_start_transpose(eT, e_sb)
                state[qi] = (r, eT)

            def stage2(qi):
                r, eT = state.pop(qi)
                op = opsum_pool.tile([P, d], f32, name="opsum")
                for sk in range(NST):
                    for c2 in range(NDCH):
                        cs = slice(c2 * DCH, (c2 + 1) * DCH)
                        nc.tensor.matmul(
                            op[:, cs],
                            lhsT=eT[:, sk, :],
                            rhs=vf_all[:, sk, cs],
                            start=(sk == 0),
                            stop=(sk == NST - 1),
                        )
                # split scale+store in half-tiles so the final drain is short
                o_sb = osb_pool.tile([P, d], f32, name="o_sb")
                H = P // 2
                for hh in range(2):
                    psl = slice(hh * H, (hh + 1) * H)
                    nc.vector.tensor_scalar_mul(o_sb[psl, :], op[psl, :], r[psl, :])
                    nc.scalar.dma_start(
                        out=out[qi * P + hh * H : qi * P + (hh + 1) * H, :],
                        in_=o_sb[psl, :],
                    )

            # PE program: Q-c0 transposes, then chunk-interleaved stage1.
            def q_xpose_chunk(c):
                for r in range(TPC):
                    pe_xpose_tile(qT_all, c, r, qf16_chunks[c][:, r, :])

            def v_xpose_half(c, half):
                for r in range(2 * half, 2 * half + 2):
                    si = c * TPC + r
                    pe_xpose_tile(vT_all, c, r, vf_all[:, si, :])

            q_xpose_chunk(0)
            ucount = {qi: 0 for qi in range(NQT)}
            for r0 in range(0, NQT, rnd):
                rr = r0 // rnd
                if rr > 0:
                    q_xpose_chunk(rr)
                for j in range(NJ):
                    if rr == 0:
                        v_xpose_half(j, 0)
                        v_xpose_half(j, 1)
                    for qi in range(r0, r0 + rnd):
                        unit(qi, j)
                        ucount[qi] += 1
                        if ucount[qi] == NJ:
                            finish_stage1(qi)
                            done.append(qi)
                            while len(done) - emitted2 > lag:
                                stage2(done[emitted2])
                                emitted2 += 1
            while emitted2 < len(done):
                stage2(done[emitted2])
                emitted2 += 1

    import os

    if bool(int(os.environ.get("KERNEL_SELF_LDW", "1"))):
        _strip_all_ldweights(nc)
    if compile:
        nc.compile()
    return nc


_CACHE = {}


def _get_nc():
    if "nc" not in _CACHE:
        _CACHE["nc"] = build_attention_nc()
    return _CACHE["nc"]


def _install_trace_support():
    """Synthesize the antenv.axon_hooks module (absent in this image) and
    register the NTFF profile hook + disable the S3 artifact upload."""
    import types
    import antenv

    if "antenv.axon_hooks" not in sys.modules:
        mod = types.ModuleType("antenv.axon_hooks")
        mod._hook = None

        def set_axon_ntff_profile_hook(h):
            mod._hook = h

        def get_axon_ntff_profile_hook():
            return mod._hook

        mod.set_axon_ntff_profile_hook = set_axon_ntff_profile_hook
        mod.get_axon_ntff_profile_hook = get_axon_ntff_profile_hook
        sys.modules["antenv.axon_hooks"] = mod
        antenv.axon_hooks = mod

    mod = sys.modules["antenv.axon_hooks"]
    if mod._hook is None:
        from trn_agent_boot.trn_boot import _ntff_profile_via_ctypes

        mod._hook = _ntff_profile_via_ctypes("/opt/axon/libaxon_pjrt.so")

    import concourse.bass_utils as bu

    bu.upload_artifacts = lambda tmpdir: tmpdir


def kernel(query: np.ndarray, value: np.ndarray) -> np.ndarray:
    from concourse.bass_utils import run_bass_kernel_spmd
    import os

    assert query.shape == (B, SQ, D) and value.shape == (B, SV, D)
    nc = _get_nc()
    in_maps = [
        {
            "q": np.ascontiguousarray(query[b], dtype=np.float32),
            "v": np.ascontiguousarray(value[b], dtype=np.float32),
        }
        for b in range(N_CORES)
    ]
    trace = bool(int(os.environ.get("KERNEL_TRACE", "0")))
    kwargs = {}
    if trace:
        _install_trace_support()
        tdir = os.environ.get("KERNEL_TRACE_DIR")
        if tdir:
            os.makedirs(tdir, exist_ok=True)
            kwargs["tmpdir"] = tdir
    res = run_bass_kernel_spmd(
        nc, in_maps, core_ids=list(range(N_CORES)), trace=trace, **kwargs
    )
    if trace:
        _CACHE["last_results"] = res
    return np.stack([res.results[b]["out"] for b in range(N_CORES)], axis=0)


# revision 15
# speedup vs baseline: 1.0622x; 1.0622x over previous
"""Bass/Trainium2 kernel for nn_AttentionLayer (B=8, SQ=SV=2048, D=1024, fp32).

attention = softmax(Q @ V^T) @ V, per batch element.

Strategy (v3)
-------------
- Batch-parallel over 8 NeuronCores (1 batch element per core, no collectives).
- fp16 operands on TensorE (full rate), fp32 PSUM accumulation, softmax in
  fp32 (DVE row-max, ScalarE exp LUT with per-row bias + fused row-sum).
- Input staging minimizes prologue HBM traffic (the v2 bottleneck):
  * V: 8 half-wave SWDGE casts fp32->fp16 straight into SBUF (8MB HBM
    read, nothing else). V^T stripes are produced by PE-mode transposes
    (tensor.transpose via identity, ~75ns each) from SBUF while the PE
    is otherwise idle - V never round-trips through DRAM.
  * Q chunk 0: two SWDGE casts to SBUF + PE transposes (so the matmul
    stream can start at ~14us).
  * Q chunks 1-3: fp32 loads on the (idle) sync HWDGE ring, engine cast
    to fp16, then PE transposes emitted at the round boundary that
    consumes them - no DRAM image, no DMA transpose, no timing risk.
  * E (softmax numerator) transposed SBUF->SBUF per q-tile (no DRAM).
- stage1 emitted in (q-tile, s-chunk) units of 8 matmuls, round-of-4
  interleaved with the V^T transpose chunks so matmuls start as soon as
  the first chunk lands.
- All DMA transposes (eT only) ride the sync HWDGE ring exclusively.
- All matmuls converted to self-loading (explicit LDWEIGHTS stripped).
- Output scale+store split in half-tiles to shorten the drain tail.
"""

import sys

if "/opt/trn_rl_repo" not in sys.path:
    sys.path.insert(0, "/opt/trn_rl_repo")

import numpy as np

B, SQ, SV, D = 8, 2048, 2048, 1024
P = 128
N_CORES = 8


def _strip_all_ldweights(nc):
    """Remove every InstLdweights, migrating its semaphore waits/updates onto
    the next PE instruction (its paired InstMatmult)."""
    import concourse.mybir as mybir

    removed = 0
    for fn in nc.m.functions:
        for bb in fn.blocks:
            keep = []
            pending = []
            for inst in bb.instructions:
                if isinstance(inst, mybir.InstLdweights):
                    if inst.sync_info is not None:
                        pending.append(inst.sync_info)
                    removed += 1
                    continue
                if isinstance(inst, mybir.InstMatmult):
                    inst.ldweights = True
                    if pending:
                        waits, updates = [], []
                        for si in pending:
                            waits.extend(list(si.on_wait))
                            updates.extend(list(si.on_update))
                        mi = inst.sync_info
                        if mi is not None:
                            waits.extend(list(mi.on_wait))
                            updates.extend(list(mi.on_update))
                        inst.sync_info = mybir.SyncInfo(
                            on_wait=waits, on_update=updates
                        )
                        pending = []
                keep.append(inst)
            assert not pending, "dangling LDW sync with no following matmul"
            bb.instructions[:] = keep
    return removed


def build_attention_nc(sq=SQ, sv=SV, d=D, lag=5, rnd=4, compile=True):
    import concourse.bass as bass
    import concourse.mybir as mybir
    from concourse import bacc
    from concourse.tile import TileContext
    from concourse.masks import make_identity

    f32 = mybir.dt.float32
    f16 = mybir.dt.float16
    X = mybir.AxisListType.X
    Exp = mybir.ActivationFunctionType.Exp

    NQT, NST, NKT = sq // P, sv // P, d // P
    CH = 512                  # row chunk (cast/xpose granularity = psum chunk)
    NCH = sv // CH            # 4
    NJ = sv // CH
    DCH = 512
    NDCH = d // DCH
    TPC = CH // P             # tiles per chunk (4)

    nc = bacc.Bacc("TRN2", target_bir_lowering=False, debug=False)
    q = nc.dram_tensor("q", [sq, d], f32, kind="ExternalInput").ap()
    v = nc.dram_tensor("v", [sv, d], f32, kind="ExternalInput").ap()
    out = nc.dram_tensor("out", [sq, d], f32, kind="ExternalOutput").ap()

    with TileContext(nc) as tc:
        with (
            tc.tile_pool(name="dram", bufs=1, space="DRAM") as dram_pool,
            tc.tile_pool(name="resident", bufs=1) as res_pool,
            tc.tile_pool(name="qf16", bufs=3) as qf16_pool,
            tc.tile_pool(name="ssb", bufs=rnd) as ssb_pool,
            tc.tile_pool(name="esb", bufs=2) as esb_pool,
            tc.tile_pool(name="etp", bufs=lag + 2) as et_pool,
            tc.tile_pool(name="osb", bufs=2) as osb_pool,
            tc.tile_pool(name="small", bufs=3 * (lag + 3)) as small_pool,
            tc.tile_pool(name="xpsum", bufs=2, space="PSUM") as xp_pool,
            tc.tile_pool(name="spsum", bufs=2, space="PSUM") as sp_pool,
            tc.tile_pool(name="opsum", bufs=2, space="PSUM") as opsum_pool,
        ):
            # Residents:
            vf_all = res_pool.tile([P, NST, d], f16, name="vf_all")
            vT_all = res_pool.tile([P, NCH, NKT, CH], f16, name="vT_all")
            qT_all = res_pool.tile([P, NCH, NKT, CH], f16, name="qT_all")
            ident = res_pool.tile([P, P], f16, name="ident")
            make_identity(nc, ident)

            # ---- SWDGE ring (gpsimd): interleaved Q-c0 + V half-wave casts
            def swdge_cast_rows(dst_sbuf, src_dram, r0, r1):
                src = src_dram[r0 * P : r1 * P, :].rearrange(
                    "(si p) d -> p si d", p=P
                )
                nc.gpsimd.dma_start(out=dst_sbuf, in_=src)

            qf16_c0 = qf16_pool.tile([P, TPC, d], f16, name="qf16")
            # ring order: qc0a, vh0, qc0b, vh1..vh7 - Q tile 0 and V chunk 0
            # land first so PE transposes + the first units start ASAP
            swdge_cast_rows(qf16_c0[:, 0:2, :], q, 0, 2)
            swdge_cast_rows(vf_all[:, 0:2, :], v, 0, 2)
            swdge_cast_rows(qf16_c0[:, 2:4, :], q, 2, 4)
            for h in range(1, 8):
                swdge_cast_rows(vf_all[:, 2 * h : 2 * h + 2, :], v, 2 * h, 2 * h + 2)
            # Q chunks 1-3: SWDGE casts straight to SBUF, queued after V so
            # they never steal early HBM bandwidth; the FIFO ring makes their
            # arrival deterministic (~50-65us, well before rounds 2-4 need
            # them for PE transposing).
            qf16_chunks = {0: qf16_c0}
            for c in range(1, NCH):
                t16 = qf16_pool.tile([P, TPC, d], f16, name="qf16")
                swdge_cast_rows(t16, q, c * TPC, (c + 1) * TPC)
                qf16_chunks[c] = t16

            # ---- PE transposes: src [128,128] f16 SBUF -> PSUM f16 --------
            # One PSUM tile gathers all 8 d-stripes of one 128-row tile;
            # a single strided DVE copy scatters them into the resident.
            def pe_xpose_tile(dst_all, c, r, src_tile_ap):
                ps = xp_pool.tile([P, NKT * P], f16, name="xpsum")
                for dk in range(NKT):
                    nc.tensor.transpose(
                        ps[:, dk * P : (dk + 1) * P],
                        src_tile_ap[:, dk * P : (dk + 1) * P],
                        ident,
                    )
                nc.vector.tensor_copy(
                    out=dst_all[:, c, :, r * P : (r + 1) * P],
                    in_=ps.rearrange("p (dk r) -> p dk r", dk=NKT),
                )


            # ---------------- main pipeline --------------------------------
            s_sb = {}
            state = {}
            done = []
            emitted2 = 0

            def unit(qi, j):
                sp = sp_pool.tile([P, CH], f32, name="spsum")
                cq, rq = qi // TPC, qi % TPC
                for dk in range(NKT):
                    nc.tensor.matmul(
                        sp,
                        lhsT=qT_all[:, cq, dk, rq * P : (rq + 1) * P],
                        rhs=vT_all[:, j, dk, :],
                        start=(dk == 0),
                        stop=(dk == NKT - 1),
                    )
                if qi not in s_sb:
                    s_sb[qi] = ssb_pool.tile([P, sv], f32, name="s_sb")
                dst = s_sb[qi][:, j * CH : (j + 1) * CH]
                if j % 2 == 0:
                    nc.scalar.copy(dst, sp)
                else:
                    nc.vector.tensor_copy(out=dst, in_=sp)

            def finish_stage1(qi):
                s = s_sb.pop(qi)
                negm = small_pool.tile([P, 1], f32, name="negm")
                nc.vector.reduce_max(negm, s, axis=X, negate=True)
                e_sb = esb_pool.tile([P, sv], f16, name="e_sb")
                lsum = small_pool.tile([P, 1], f32, name="lsum")
                nc.scalar.activation(
                    e_sb, s, Exp, bias=negm, scale=1.0, accum_out=lsum
                )
                r = small_pool.tile([P, 1], f32, name="r")
                nc.vector.reciprocal(r, lsum)
                eT = et_pool.tile([P, NST, P], f16, name="eT")
                nc.sync.dma_start_transpose(eT, e_sb)
                state[qi] = (r, eT)

            def stage2(qi):
                r, eT = state.pop(qi)
                op = opsum_pool.tile([P, d], f32, name="opsum")
                for sk in range(NST):
                    for c2 in range(NDCH):
                        cs = slice(c2 * DCH, (c2 + 1) * DCH)
                        nc.tensor.matmul(
                            op[:, cs],
                            lhsT=eT[:, sk, :],
                            rhs=vf_all[:, sk, cs],
                            start=(sk == 0),
                            stop=(sk == NST - 1),
                        )
                # split scale+store in half-tiles so the final drain is short
                o_sb = osb_pool.tile([P, d], f32, name="o_sb")
                H = P // 2
                for hh in range(2):
                    psl = slice(hh * H, (hh + 1) * H)
                    nc.vector.tensor_scalar_mul(o_sb[psl, :], op[psl, :], r[psl, :])
                    nc.scalar.dma_start(
                        out=out[qi * P + hh * H : qi * P + (hh + 1) * H, :],
                        in_=o_sb[psl, :],
                    )

            # PE program: Q-c0 transposes, then chunk-interleaved stage1.
            def q_xpose_chunk(c):
                for r in range(TPC):
                    pe_xpose_tile(qT_all, c, r, qf16_chunks[c][:, r, :])

            def v_xpose_half(c, half):
                for r in range(2 * half, 2 * half + 2):
                    si = c * TPC + r
                    pe_xpose_tile(vT_all, c, r, vf_all[:, si, :])

            q_xpose_chunk(0)
            ucount = {qi: 0 for qi in range(NQT)}
            for r0 in range(0, NQT, rnd):
                rr = r0 // rnd
                if rr > 0:
                    q_xpose_chunk(rr)
                for j in range(NJ):
                    if rr == 0:
                        v_xpose_half(j, 0)
                        v_xpose_half(j, 1)
                    for qi in range(r0, r0 + rnd):
                        unit(qi, j)
                        ucount[qi] += 1
                        if ucount[qi] == NJ:
                            finish_stage1(qi)
                            done.append(qi)
                            while len(done) - emitted2 > lag:
                                stage2(done[emitted2])
                                emitted2 += 1
            while emitted2 < len(done):
                stage2(done[emitted2])
                emitted2 += 1

    import os

    if bool(int(os.environ.get("KERNEL_SELF_LDW", "1"))):
        _strip_all_ldweights(nc)
    if compile:
        nc.compile()
    return nc


_CACHE = {}


def _get_nc():
    if "nc" not in _CACHE:
        _CACHE["nc"] = build_attention_nc()
    return _CACHE["nc"]


def _install_trace_support():
    """Synthesize the antenv.axon_hooks module (absent in this image) and
    register the NTFF profile hook + disable the S3 artifact upload."""
    import types
    import antenv

    if "antenv.axon_hooks" not in sys.modules:
        mod = types.ModuleType("antenv.axon_hooks")
        mod._hook = None

        def set_axon_ntff_profile_hook(h):
            mod._hook = h

        def get_axon_ntff_profile_hook():
            return mod._hook

        mod.set_axon_ntff_profile_hook = set_axon_ntff_profile_hook
        mod.get_axon_ntff_profile_hook = get_axon_ntff_profile_hook
        sys.modules["antenv.axon_hooks"] = mod
        antenv.axon_hooks = mod

    mod = sys.modules["antenv.axon_hooks"]
    if mod._hook is None:
        from trn_agent_boot.trn_boot import _ntff_profile_via_ctypes

        mod._hook = _ntff_profile_via_ctypes("/opt/axon/libaxon_pjrt.so")

    import concourse.bass_utils as bu

    bu.upload_artifacts = lambda tmpdir: tmpdir


def kernel(query: np.ndarray, value: np.ndarray) -> np.ndarray:
    from concourse.bass_utils import run_bass_kernel_spmd
    import os

    assert query.shape == (B, SQ, D) and value.shape == (B, SV, D)
    nc = _get_nc()
    in_maps = [
        {
            "q": np.ascontiguousarray(query[b], dtype=np.float32),
            "v": np.ascontiguousarray(value[b], dtype=np.float32),
        }
        for b in range(N_CORES)
    ]
    trace = bool(int(os.environ.get("KERNEL_TRACE", "0")))
    kwargs = {}
    if trace:
        _install_trace_support()
        tdir = os.environ.get("KERNEL_TRACE_DIR")
        if tdir:
            os.makedirs(tdir, exist_ok=True)
            kwargs["tmpdir"] = tdir
    res = run_bass_kernel_spmd(
        nc, in_maps, core_ids=list(range(N_CORES)), trace=trace, **kwargs
    )
    if trace:
        _CACHE["last_results"] = res
    return np.stack([res.results[b]["out"] for b in range(N_CORES)], axis=0)
